# revision 1
# baseline (speedup 1.0000x reference)
"""Causal multi-head self-attention (RoPE) Trainium2 Bass kernel.

Sharding: 8 cores = 2 batches x 4 head-groups (4 heads each).
Per core: QKV projections for its head slice, RoPE, causal flash-style
attention (S^T orientation, ones-row softmax denominator), O-projection
partial, then a chunked ReduceScatter over the 4 cores of each batch.
All matmuls run as float32r (TF32-like, fp32 accumulate).
"""

import sys, math

sys.path.insert(0, '/opt/trn_rl_repo')
import numpy as np

B, S, D, H, DK = 2, 2048, 1024, 16, 64
HC = 4            # heads per core
E = HC * DK       # 256 local projection width
NSC = S // 128    # 16 s-chunks
NQT = S // 256    # 8 q-tiles of 256
ROPE_THETA = 10000.0
MASK_VAL = -1e30
C1 = 6.28125                  # Cody-Waite split of 2*pi
C2 = 2 * math.pi - C1

_compiled = None
DEBUG = False


def _build():
    import concourse.bass as bass
    import concourse.tile as tile
    from concourse import bacc, mybir
    from concourse.masks import make_identity

    F32 = mybir.dt.float32
    F32R = mybir.dt.float32r
    I32 = mybir.dt.int32
    AF = mybir.ActivationFunctionType
    ALU = mybir.AluOpType

    nc = bacc.Bacc()

    x_d = nc.dram_tensor("x", [S, D], F32, kind="ExternalInput")
    wq_d = nc.dram_tensor("wq", [D, E], F32R, kind="ExternalInput")
    wk_d = nc.dram_tensor("wk", [D, E], F32R, kind="ExternalInput")
    wv_d = nc.dram_tensor("wv", [D, E], F32R, kind="ExternalInput")
    wo_d = nc.dram_tensor("wo", [E, D], F32R, kind="ExternalInput")
    pos_d = nc.dram_tensor("pos", [S], I32, kind="ExternalInput")
    ivf_d = nc.dram_tensor("ivf", [128, 32], F32, kind="ExternalInput")
    m0_d = nc.dram_tensor("m0", [128, 256], F32, kind="ExternalInput")
    m1_d = nc.dram_tensor("m1", [128, 256], F32, kind="ExternalInput")
    y_d = nc.dram_tensor("y", [512, D], F32, kind="ExternalOutput")
    if DEBUG:
        dbg_qt = nc.dram_tensor("dbg_qt", [128, S], F32, kind="ExternalOutput")
        dbg_kt = nc.dram_tensor("dbg_kt", [128, S], F32, kind="ExternalOutput")
        dbg_vo = nc.dram_tensor("dbg_vo", [128, HC * 65], F32, kind="ExternalOutput")
        dbg_pt = nc.dram_tensor("dbg_pt", [128, 2, 256], F32, kind="ExternalOutput")
        dbg_ao = nc.dram_tensor("dbg_ao", [128, 256], F32, kind="ExternalOutput")
        dbg_aot = nc.dram_tensor("dbg_aot", [128, S], F32, kind="ExternalOutput")
        dbg_cos = nc.dram_tensor("dbg_cos", [128, 512], F32, kind="ExternalOutput")
        dbg_sin = nc.dram_tensor("dbg_sin", [128, 512], F32, kind="ExternalOutput")
    cc_in = nc.dram_tensor("cc_in", [S, D], F32)
    cc_out = nc.dram_tensor("cc_out", [512, D], F32)
    groups = [[0, 1, 2, 3], [4, 5, 6, 7]]

    with tile.TileContext(nc) as tc:
        with (
            tc.tile_pool(name="const", bufs=1) as cp,
            tc.tile_pool(name="big", bufs=1) as bp,
            tc.tile_pool(name="xs", bufs=3) as xsp,
            tc.tile_pool(name="xt", bufs=2) as xtp,
            tc.tile_pool(name="rope", bufs=2) as rp,
            tc.tile_pool(name="pt", bufs=2) as ptp,
            tc.tile_pool(name="outs", bufs=2) as osp,
            tc.tile_pool(name="small", bufs=2) as smp,
            tc.tile_pool(name="trps", bufs=2, space="PSUM") as trps,
            tc.tile_pool(name="qkvps", bufs=1, space="PSUM") as qkvps,
            tc.tile_pool(name="stps", bufs=2, space="PSUM") as stps,
            tc.tile_pool(name="aops", bufs=1, space="PSUM") as aops,
        ):
            # ---- constants / weights
            wq_t = cp.tile([128, 8, E], F32R, tag="wq")
            wk_t = cp.tile([128, 8, E], F32R, tag="wk")
            wv_t = cp.tile([128, 8, E], F32R, tag="wv")
            wo_t = cp.tile([128, 2, D], F32R, tag="wo")
            nc.sync.dma_start(wq_t[:], wq_d.rearrange("(c p) e -> p c e", p=128))
            nc.sync.dma_start(wk_t[:], wk_d.rearrange("(c p) e -> p c e", p=128))
            nc.sync.dma_start(wv_t[:], wv_d.rearrange("(c p) e -> p c e", p=128))
            nc.sync.dma_start(wo_t[:], wo_d.rearrange("(c p) e -> p c e", p=128))
            masks = [cp.tile([128, 256], F32, tag="m0", name="mask0"),
                     cp.tile([128, 256], F32, tag="m1", name="mask1")]
            nc.sync.dma_start(masks[0][:], m0_d[:])
            nc.sync.dma_start(masks[1][:], m1_d[:])
            ident = cp.tile([128, 128], F32, tag="ident")
            make_identity(nc, ident[:])
            ones_f = cp.tile([1, 64], F32, tag="ones_f")
            nc.vector.memset(ones_f[:], 1.0)
            ones_r = cp.tile([1, 64], F32R, tag="ones_r")
            nc.scalar.copy(ones_r[:], ones_f[:])

            # ---- RoPE tables: SINT/COST/NEGSINT [128, 16, 32]
            sint = cp.tile([128, NSC, 32], F32, tag="sint")
            cost = cp.tile([128, NSC, 32], F32, tag="cost")
            nsint = cp.tile([128, NSC, 32], F32, tag="nsint")
            with tc.tile_pool(name="tbl", bufs=1) as tp:
                ivf_t = tp.tile([128, 32], F32, tag="ivf")
                nc.sync.dma_start(ivf_t[:], ivf_d[:])
                pos_i = tp.tile([128, NSC], I32, tag="pos_i")
                nc.sync.dma_start(pos_i[:], pos_d.rearrange("(c p) -> p c", p=128))
                posf = tp.tile([128, NSC], F32, tag="posf")
                nc.vector.tensor_copy(posf[:], pos_i[:])
                ang = tp.tile([128, NSC, 32], F32, tag="ang")
                for c in range(NSC):
                    nc.vector.tensor_scalar(out=ang[:, c, :], in0=ivf_t[:],
                                            scalar1=posf[:, c:c + 1], scalar2=None,
                                            op0=ALU.mult)
                qf = tp.tile([128, NSC, 32], F32, tag="qf")
                ni = tp.tile([128, NSC, 32], I32, tag="ni")
                nf = tp.tile([128, NSC, 32], F32, tag="nf")
                tmp = tp.tile([128, NSC, 32], F32, tag="tmp")
                red = tp.tile([128, NSC, 32], F32, tag="red")
                inv2pi = 1.0 / (2 * math.pi)
                # sin: red = ang - round(ang/2pi)*(C1+C2)
                nc.vector.tensor_scalar(out=qf[:], in0=ang[:], scalar1=inv2pi,
                                        scalar2=None, op0=ALU.mult)
                nc.vector.tensor_copy(ni[:], qf[:])
                nc.vector.tensor_copy(nf[:], ni[:])
                nc.vector.tensor_scalar(out=tmp[:], in0=nf[:], scalar1=-C1,
                                        scalar2=None, op0=ALU.mult)
                nc.vector.tensor_tensor(out=red[:], in0=ang[:], in1=tmp[:], op=ALU.add)
                nc.vector.tensor_scalar(out=tmp[:], in0=nf[:], scalar1=-C2,
                                        scalar2=None, op0=ALU.mult)
                nc.vector.tensor_tensor(out=red[:], in0=red[:], in1=tmp[:], op=ALU.add)
                nc.scalar.activation(sint[:], red[:], AF.Sin)
                # cos(x) = sin(x + pi/2): second reduction
                nc.vector.tensor_scalar(out=qf[:], in0=ang[:], scalar1=inv2pi,
                                        scalar2=0.25, op0=ALU.mult, op1=ALU.add)
                nc.vector.tensor_copy(ni[:], qf[:])
                nc.vector.tensor_copy(nf[:], ni[:])
                nc.vector.tensor_scalar(out=tmp[:], in0=nf[:], scalar1=-C1,
                                        scalar2=None, op0=ALU.mult)
                nc.vector.tensor_tensor(out=red[:], in0=ang[:], in1=tmp[:], op=ALU.add)
                nc.vector.tensor_scalar(out=tmp[:], in0=nf[:], scalar1=-C2,
                                        scalar2=math.pi / 2, op0=ALU.mult, op1=ALU.add)
                nc.vector.tensor_tensor(out=red[:], in0=red[:], in1=tmp[:], op=ALU.add)
                nc.scalar.activation(cost[:], red[:], AF.Sin)
                nc.vector.tensor_scalar(out=nsint[:], in0=sint[:], scalar1=-1.0,
                                        scalar2=None, op0=ALU.mult)

            # ---- persistent activation tensors
            QT = [bp.tile([128, S], F32R, tag=f"qt{i}", name=f"qt{i}") for i in range(2)]
            KT = [bp.tile([128, S], F32R, tag=f"kt{i}", name=f"kt{i}") for i in range(2)]
            VO = bp.tile([128, NSC, HC * 65], F32R, tag="vo")
            AOT = [bp.tile([128, S], F32R, tag=f"aot{i}", name=f"aot{i}") for i in range(2)]
            # ones column of VO (softmax denominator trick)
            ones_col = cp.tile([128, HC], F32, tag="ones_col")
            nc.vector.memset(ones_col[:], 1.0)
            for sc in range(NSC):
                nc.vector.tensor_copy(
                    VO[:].rearrange("p c (h z) -> p c h z", z=65)[:, sc, :, 64:65],
                    ones_col[:].rearrange("p (h a) -> p h a", a=1))

            # ================= Phase A: QKV + RoPE + transposes =================
            for sc in range(NSC):
                xs = xsp.tile([128, D], F32, tag="xs")
                nc.sync.dma_start(xs[:], x_d[sc * 128:(sc + 1) * 128, :])
                xt = xtp.tile([128, 8, 128], F32R, tag="xt")
                for dc in range(8):
                    tp_ps = trps.tile([128, 128], F32, tag="tr")
                    nc.tensor.transpose(tp_ps[:], xs[:, dc * 128:(dc + 1) * 128],
                                        ident[:])
                    nc.scalar.copy(xt[:, dc, :], tp_ps[:])
                q_ps = qkvps.tile([128, E], F32, tag="qp")
                k_ps = qkvps.tile([128, E], F32, tag="kp")
                v_ps = qkvps.tile([128, E], F32, tag="vp")
                qkv = [q_ps, k_ps, v_ps]
                for dc in range(8):
                    nc.tensor.matmul(q_ps[:], xt[:, dc, :], wq_t[:, dc, :],
                                     start=(dc == 0), stop=(dc == 7))
                    nc.tensor.matmul(k_ps[:], xt[:, dc, :], wk_t[:, dc, :],
                                     start=(dc == 0), stop=(dc == 7))
                    nc.tensor.matmul(v_ps[:], xt[:, dc, :], wv_t[:, dc, :],
                                     start=(dc == 0), stop=(dc == 7))
                cosb = cost[:, sc, :].rearrange("p (a f) -> p a f", a=1) \
                    .to_broadcast([128, 8, 32])
                sinb = sint[:, sc, :].rearrange("p (a f) -> p a f", a=1) \
                    .to_broadcast([128, 4, 32])
                nsinb = nsint[:, sc, :].rearrange("p (a f) -> p a f", a=1) \
                    .to_broadcast([128, 4, 32])
                for ti, dst in ((0, QT), (1, KT)):
                    src = qkv[ti][:]
                    t_s = rp.tile([128, E], F32, tag="t")
                    u_s = rp.tile([128, E], F32, tag="u")
                    nc.vector.tensor_tensor(
                        out=t_s[:].rearrange("p (a f) -> p a f", f=32),
                        in0=src.rearrange("p (a f) -> p a f", f=32),
                        in1=cosb, op=ALU.mult)
                    s4 = src.rearrange("p (h two f) -> p h two f", two=2, f=32)
                    u4 = u_s[:].rearrange("p (h two f) -> p h two f", two=2, f=32)
                    nc.vector.tensor_tensor(out=u4[:, :, 0, :], in0=s4[:, :, 1, :],
                                            in1=nsinb, op=ALU.mult)
                    nc.vector.tensor_tensor(out=u4[:, :, 1, :], in0=s4[:, :, 0, :],
                                            in1=sinb, op=ALU.mult)
                    for half in range(2):
                        tr2 = trps.tile([128, 128], F32, tag="tr")
                        nc.tensor.transpose(
                            tr2[:], t_s[:, half * 128:(half + 1) * 128], ident[:])
                        nc.tensor.matmul(
                            tr2[:], u_s[:, half * 128:(half + 1) * 128], ident[:],
                            is_transpose=True, start=False, stop=True)
                        nc.scalar.copy(dst[half][:, sc * 128:(sc + 1) * 128], tr2[:])
                # V: strided copy into 65-wide head groups
                nc.vector.tensor_copy(
                    VO[:].rearrange("p c (h z) -> p c h z", z=65)[:, sc, :, 0:64],
                    v_ps[:].rearrange("p (h f) -> p h f", f=64))

            # ================= Phase B: attention + O-proj =================
            for qt in range(NQT):
                for h in range(HC):
                    hc, hb = h // 2, (h % 2) * 64
                    kmax = 2 * qt + 1
                    ao = aops.tile([128, 256], F32, tag="ao")
                    pt = ptp.tile([128, NSC, 256], F32R, tag="pt")
                    for kc2 in range(0, kmax + 1, 2):
                        st = stps.tile([128, 512], F32, tag="st")
                        for j in range(2):
                            kc = kc2 + j
                            sl = st[:, j * 256:(j + 1) * 256]
                            nc.tensor.matmul(
                                sl,
                                KT[hc][hb:hb + 64, kc * 128:(kc + 1) * 128],
                                QT[hc][hb:hb + 64, qt * 256:(qt + 1) * 256],
                                start=True, stop=True)
                            if kc >= 2 * qt:
                                nc.vector.tensor_tensor(out=sl, in0=sl,
                                                        in1=masks[kc - 2 * qt][:],
                                                        op=ALU.add)
                        nc.scalar.activation(pt[:, kc2:kc2 + 2, :], st[:],
                                             AF.Exp, scale=1.0 / math.sqrt(DK))
                        for j in range(2):
                            kc = kc2 + j
                            nc.tensor.matmul(
                                ao[0:65, :],
                                VO[:, kc, h * 65:(h + 1) * 65],
                                pt[:, kc, :],
                                start=(kc == 0), stop=(kc == kmax))
                    rec = smp.tile([1, 256], F32R, tag="rec")
                    with nc.allow_low_precision(reason="f32r softmax denom"):
                        nc.vector.reciprocal(rec[:], ao[64:65, :])
                    rep = trps.tile([64, 256], F32, tag="tr")
                    nc.tensor.matmul(rep[:], ones_r[:], rec[:], start=True, stop=True)
                    rep_sb = smp.tile([64, 256], F32, tag="rep_sb")
                    nc.scalar.copy(rep_sb[:], rep[:])
                    with nc.allow_low_precision(reason="f32r attention output"):
                        nc.vector.tensor_tensor(
                            out=AOT[hc][hb:hb + 64, qt * 256:(qt + 1) * 256],
                            in0=ao[0:64, :], in1=rep_sb[:], op=ALU.mult)
                # O-projection for the two s-chunks of this q-tile
                for scl in (2 * qt, 2 * qt + 1):
                    outs = osp.tile([128, D], F32, tag="outs")
                    for nb in range(2):
                        op = stps.tile([128, 512], F32, tag="st")
                        for cc in range(2):
                            nc.tensor.matmul(
                                op[:],
                                AOT[cc][:, scl * 128:(scl + 1) * 128],
                                wo_t[:, cc, nb * 512:(nb + 1) * 512],
                                start=(cc == 0), stop=(cc == 1))
                        nc.scalar.copy(outs[:, nb * 512:(nb + 1) * 512], op[:])
                    nc.sync.dma_start(cc_in[scl * 128:(scl + 1) * 128, :], outs[:])
                # chunked reduce-scatter every 2 q-tiles (512 rows ready)
                if qt % 2 == 1:
                    jj = qt // 2
                    nc.gpsimd.collective_compute(
                        "ReduceScatter", ALU.add, replica_groups=groups,
                        ins=[cc_in[512 * jj:512 * (jj + 1), :]],
                        outs=[cc_out[128 * jj:128 * (jj + 1), :]])
                    nc.sync.dma_start(y_d[128 * jj:(jj + 1) * 128, :],
                                      cc_out[128 * jj:128 * (jj + 1), :])
            if DEBUG:
                nc.sync.dma_start(dbg_qt[:], QT[0][:].bitcast(F32))
                nc.sync.dma_start(dbg_kt[:], KT[0][:].bitcast(F32))
                nc.sync.dma_start(dbg_vo[:], VO[:, 0, :].bitcast(F32))
                nc.sync.dma_start(dbg_aot[:], AOT[0][:].bitcast(F32))
                nc.sync.dma_start(dbg_cos[:], cost[:].rearrange("p c f -> p (c f)"))
                nc.sync.dma_start(dbg_sin[:], sint[:].rearrange("p c f -> p (c f)"))

    nc.compile()
    return nc


def _get_compiled():
    global _compiled
    if _compiled is None:
        _compiled = _build()
    return _compiled


def kernel(x, Wq, Wk, Wv, Wo, token_positions):
    from concourse.bass_utils import run_bass_kernel_spmd

    nc = _get_compiled()

    x = np.asarray(x, np.float32)
    Wq = np.asarray(Wq, np.float32)
    Wk = np.asarray(Wk, np.float32)
    Wv = np.asarray(Wv, np.float32)
    Wo = np.asarray(Wo, np.float32)
    pos = np.ascontiguousarray(np.asarray(token_positions).astype(np.int32))

    # rotate-half permutation within each head: [evens, odds]
    perm = np.concatenate([np.arange(0, DK, 2), np.arange(1, DK, 2)])
    inv_freq = (ROPE_THETA ** (-np.arange(0, DK, 2, dtype=np.float32) / DK)
                ).astype(np.float32)
    ivf = np.ascontiguousarray(np.broadcast_to(inv_freq[None, :], (128, 32))
                               ).astype(np.float32)
    kl = np.arange(128)[:, None]
    ql = np.arange(256)[None, :]
    m0 = np.where(kl <= ql, 0.0, MASK_VAL).astype(np.float32)
    m1 = np.where(kl + 128 <= ql, 0.0, MASK_VAL).astype(np.float32)

    in_maps = []
    for c in range(8):
        b, g = c // 4, c % 4
        heads = range(HC * g, HC * (g + 1))
        rowsel = np.concatenate([h * DK + perm for h in heads])
        block = slice(E * g, E * (g + 1))
        in_maps.append({
            "x": np.ascontiguousarray(x[b]),
            "wq": np.ascontiguousarray(Wq[rowsel, :].T),
            "wk": np.ascontiguousarray(Wk[rowsel, :].T),
            "wv": np.ascontiguousarray(Wv[block, :].T),
            "wo": np.ascontiguousarray(Wo[:, block].T),
            "pos": pos, "ivf": ivf, "m0": m0, "m1": m1,
        })

    res = run_bass_kernel_spmd(nc, in_maps, core_ids=list(range(8)))

    out = np.empty((B, S, D), np.float32)
    for b in range(B):
        for j in range(4):
            for r in range(4):
                shard = res.results[4 * b + r]["y"]
                out[b, 512 * j + 128 * r: 512 * j + 128 * (r + 1), :] = \
                    shard[128 * j:128 * (j + 1), :]
    return out



# revision 10
# speedup vs baseline: 1.6001x; 1.6001x over previous
"""Causal multi-head self-attention (RoPE) Trainium2 Bass kernel.

Sharding: 8 cores = 2 batches x 4 head-groups (4 heads each).
Per core: QKV projections for its head slice, RoPE, causal flash-style
attention (S^T orientation, ones-row softmax denominator), O-projection
partial, then a per-qtile ReduceScatter (bf16) over the 4 cores of each
batch.

Fast path vs v1: bf16 operands everywhere (fp32 PSUM accumulation),
host-side x transpose + cos/sin tables, phase A/B interleaving to keep
the PE warm (HAM), row-tiled K=64 S^T matmuls (two heads concurrent in
PE quadrants), exp on [128,1024] psum tiles, multiplicative causal mask
on GPSIMD, reciprocal_approx_fast + K=1 broadcast matmul for softmax
denominators.
"""

import sys, math

sys.path.insert(0, '/opt/trn_rl_repo')
import numpy as np

B, S, D, H, DK = 2, 2048, 1024, 16, 64
HC = 4            # heads per core
E = HC * DK       # 256 local projection width
NSC = S // 128    # 16 s-chunks
NQT = S // 256    # 8 q-tiles of 256
ROPE_THETA = 10000.0

_compiled = None
import os
DEBUG = bool(int(os.environ.get("KDEBUG", "0")))


def _build():
    import concourse.bass as bass
    import concourse.tile as tile
    from concourse import bacc, mybir
    from concourse.masks import make_identity

    F32 = mybir.dt.float32
    F32R = mybir.dt.float32r
    BF = mybir.dt.bfloat16
    AF = mybir.ActivationFunctionType
    ALU = mybir.AluOpType

    nc = bacc.Bacc()

    xt_d = nc.dram_tensor("xt", [D, S], BF, kind="ExternalInput")
    wqk_d = nc.dram_tensor("wqk", [D, 2 * E], BF, kind="ExternalInput")
    wv_d = nc.dram_tensor("wv", [D, E], BF, kind="ExternalInput")
    wo_d = nc.dram_tensor("wo", [E, D], BF, kind="ExternalInput")
    cos_d = nc.dram_tensor("cost", [128, NSC * 32], BF, kind="ExternalInput")
    sin_d = nc.dram_tensor("sint", [128, NSC * 32], BF, kind="ExternalInput")
    nsin_d = nc.dram_tensor("nsint", [128, NSC * 32], BF, kind="ExternalInput")
    msk_d = nc.dram_tensor("msk", [128, 2 * 256], BF, kind="ExternalInput")
    y_d = nc.dram_tensor("y", [512, D], BF, kind="ExternalOutput")
    if DEBUG:
        dbg_qtkt = nc.dram_tensor("dbg_qtkt", [128, 4 * S], BF, kind="ExternalOutput")
        dbg_vo = nc.dram_tensor("dbg_vo", [128, NSC * HC * 65], BF, kind="ExternalOutput")
        dbg_aot = nc.dram_tensor("dbg_aot", [128, 2 * S], BF, kind="ExternalOutput")
        dbg_pt = nc.dram_tensor("dbg_pt", [128, 2 * NSC * 256], BF, kind="ExternalOutput")
        dbg_st = nc.dram_tensor("dbg_st", [128, 1024], mybir.dt.float32, kind="ExternalOutput")
        dbg_ao = nc.dram_tensor("dbg_ao", [65, 1024], mybir.dt.float32, kind="ExternalOutput")
        dbg_rep = nc.dram_tensor("dbg_rep", [128, 1024], mybir.dt.float32, kind="ExternalOutput")
    cc_in = nc.dram_tensor("cc_in", [S, D], BF)
    cc_out = nc.dram_tensor("cc_out", [512, D], BF)
    groups = [[0, 1, 2, 3], [4, 5, 6, 7]]

    with tile.TileContext(nc) as tc:
        with (
            tc.tile_pool(name="const", bufs=1) as cp,
            tc.tile_pool(name="persist", bufs=1) as bp,
            tc.tile_pool(name="rope", bufs=2) as rp,
            tc.tile_pool(name="pt", bufs=2) as ptp,
            tc.tile_pool(name="rec", bufs=2) as rcp,
            tc.tile_pool(name="stage", bufs=3) as stg,
            tc.tile_pool(name="big", bufs=2, space="PSUM") as bigp,
            tc.tile_pool(name="trp", bufs=2, space="PSUM") as trp,
            tc.tile_pool(name="aop", bufs=1, space="PSUM") as aop,
        ):
            # ---- constants / weights (bf16)
            wqk_t = cp.tile([128, 8, 2 * E], BF, tag="wqk")
            wv_t = cp.tile([128, 8, E], BF, tag="wv")
            wo_t = cp.tile([128, 2, D], BF, tag="wo")
            nc.sync.dma_start(wqk_t[:], wqk_d.rearrange("(c p) e -> p c e", p=128))
            nc.sync.dma_start(wv_t[:], wv_d.rearrange("(c p) e -> p c e", p=128))
            nc.sync.dma_start(wo_t[:], wo_d.rearrange("(c p) e -> p c e", p=128))
            cost = cp.tile([128, NSC, 32], BF, tag="cost")
            sint = cp.tile([128, NSC, 32], BF, tag="sint")
            nsint = cp.tile([128, NSC, 32], BF, tag="nsint")
            nc.sync.dma_start(cost[:].rearrange("p c f -> p (c f)"), cos_d[:])
            nc.sync.dma_start(sint[:].rearrange("p c f -> p (c f)"), sin_d[:])
            nc.sync.dma_start(nsint[:].rearrange("p c f -> p (c f)"), nsin_d[:])
            masks = cp.tile([128, 2, 256], BF, tag="masks")
            nc.sync.dma_start(masks[:].rearrange("p a b -> p (a b)"), msk_d[:])
            ident = cp.tile([128, 128], BF, tag="ident")
            make_identity(nc, ident[:])
            ones_sel = cp.tile([1, 128], BF, tag="ones_sel")
            nc.vector.memset(ones_sel[:], 1.0)

            # x^T resident in SBUF: [d-part, d-chunk, s]
            xt = bp.tile([128, 8, S], BF, tag="xt")
            for c2 in range(4):
                nc.sync.dma_start(
                    xt[:, 2 * c2:2 * c2 + 2, :],
                    xt_d.rearrange("(c p) s -> p c s", p=128)[:, 2 * c2:2 * c2 + 2, :])

            # persistent activations
            # QTKT: [p, 4, S]: 0=Q feats 0:128 (heads 0,1), 1=Q feats 128:256,
            #                  2=K feats 0:128, 3=K feats 128:256
            QTKT = bp.tile([128, 4, S], BF, tag="qtkt")
            VO = bp.tile([128, NSC, HC, 65], BF, tag="vo")
            AOT = [bp.tile([128, S], BF, tag=f"aot{i}", name=f"aot{i}")
                   for i in range(2)]
            nc.vector.memset(VO[:, :, :, 64:65], 1.0)

            def phase_a(sc):
                qkp = bigp.tile([128, 1024], F32, tag="big", name="qkp")
                for dc in range(8):
                    nc.tensor.matmul(qkp[:, 0:512], xt[:, dc, sc * 128:(sc + 1) * 128],
                                     wqk_t[:, dc, :], start=(dc == 0), stop=(dc == 7))
                    nc.tensor.matmul(qkp[:, 512:768], xt[:, dc, sc * 128:(sc + 1) * 128],
                                     wv_t[:, dc, :], start=(dc == 0), stop=(dc == 7))
                # RoPE on q|k together: [128, 512]
                cosb = cost[:, sc, :].rearrange("p (a f) -> p a f", a=1) \
                    .to_broadcast([128, 16, 32])
                sinb = sint[:, sc, :].rearrange("p (a f) -> p a f", a=1) \
                    .to_broadcast([128, 8, 32])
                nsinb = nsint[:, sc, :].rearrange("p (a f) -> p a f", a=1) \
                    .to_broadcast([128, 8, 32])
                t2 = rp.tile([128, 512], BF, tag="t2")
                u2 = rp.tile([128, 512], BF, tag="u2")
                src = qkp[:, 0:512]
                with nc.allow_low_precision(reason="bf16 rope"):
                    nc.vector.tensor_tensor(
                        out=t2[:].rearrange("p (a f) -> p a f", f=32),
                        in0=src.rearrange("p (a f) -> p a f", f=32),
                        in1=cosb, op=ALU.mult)
                    s4 = src.rearrange("p (h two f) -> p h two f", two=2, f=32)
                    u4 = u2[:].rearrange("p (h two f) -> p h two f", two=2, f=32)
                    nc.vector.tensor_tensor(out=u4[:, :, 0, :], in0=s4[:, :, 1, :],
                                            in1=nsinb, op=ALU.mult)
                    nc.vector.tensor_tensor(out=u4[:, :, 1, :], in0=s4[:, :, 0, :],
                                            in1=sinb, op=ALU.mult)
                    # t2 += u2 (bf16 psum transpose cannot accumulate)
                    nc.vector.tensor_tensor(out=t2[:], in0=t2[:], in1=u2[:],
                                            op=ALU.add)
                tr = trp.tile([128, 512], BF, tag="tr", name="tr_a")
                for q in range(4):
                    nc.tensor.transpose(tr[:, q * 128:(q + 1) * 128],
                                        t2[:, q * 128:(q + 1) * 128], ident[:])
                with nc.allow_low_precision(reason="bf16 store"):
                    nc.vector.tensor_copy(
                        QTKT[:, :, sc * 128:(sc + 1) * 128],
                        tr[:].rearrange("p (a f) -> p a f", f=128))
                    # V into 65-wide head groups
                    nc.vector.tensor_copy(
                        VO[:, sc, :, 0:64],
                        qkp[:, 512:768].rearrange("p (h f) -> p h f", f=64))

            def phase_b(m):
                nkc = 2 * m + 2
                for p in range(2):  # head pair: heads (2p, 2p+1) locally
                    pt = ptp.tile([128, 2, NSC, 256], BF, tag="pt")
                    ao = aop.tile([65, 1024], F32, tag="ao")
                    if DEBUG and m == 0 and p == 0:
                        DBG["pt"] = pt
                    for kc2 in range(0, nkc, 2):
                        st = bigp.tile([128, 1024], F32, tag="big", name="st")
                        for j in range(2):
                            kc = kc2 + j
                            # row-tiled pair: head 2p on rows 0:64 -> bank0,
                            # head 2p+1 on rows 64:128 -> bank1 (concurrent)
                            nc.tensor.matmul(
                                st[:, j * 256:(j + 1) * 256],
                                QTKT[0:64, 2 + p, kc * 128:(kc + 1) * 128],
                                QTKT[0:64, p, m * 256:(m + 1) * 256],
                                start=True, stop=True)
                            nc.tensor.matmul(
                                st[:, 512 + j * 256:512 + (j + 1) * 256],
                                QTKT[64:128, 2 + p, kc * 128:(kc + 1) * 128],
                                QTKT[64:128, p, m * 256:(m + 1) * 256],
                                start=True, stop=True)
                        with nc.allow_low_precision(reason="bf16 probs"):
                            nc.scalar.activation(
                                pt[:, :, kc2:kc2 + 2, :],
                                st[:].rearrange("p (h j q) -> p h j q", h=2, q=256),
                                AF.Exp, scale=1.0 / math.sqrt(DK))
                    # causal mask on the diagonal chunk-pair (multiplicative)
                    with nc.allow_low_precision(reason="bf16 mask"):
                        nc.vector.tensor_tensor(
                            out=pt[:, :, 2 * m:2 * m + 2, :],
                            in0=pt[:, :, 2 * m:2 * m + 2, :],
                            in1=masks[:].rearrange("p a b -> p () a b")
                                .to_broadcast([128, 2, 2, 256]),
                            op=ALU.mult)
                    for kc in range(nkc):
                        nc.tensor.matmul(ao[0:65, 0:256],
                                         VO[:, kc, 2 * p, :], pt[:, 0, kc, :],
                                         start=(kc == 0), stop=(kc == nkc - 1))
                        nc.tensor.matmul(ao[0:65, 512:768],
                                         VO[:, kc, 2 * p + 1, :], pt[:, 1, kc, :],
                                         start=(kc == 0), stop=(kc == nkc - 1))
                    # normalize: rec = 1/denominators, broadcast via K=1 matmul
                    if DEBUG and m == 0 and p == 0:
                        dbg_ao_sb = stg.tile([65, 1024], F32, tag="dbgao", name="dbgao")
                        nc.vector.tensor_copy(dbg_ao_sb[:], ao[0:65, :])
                        nc.sync.dma_start(dbg_ao[:], dbg_ao_sb[:])
                    d2 = rcp.tile([1, 512], F32, tag="d2")
                    rec = rcp.tile([1, 512], F32, tag="rec")
                    rec_b = rcp.tile([1, 512], BF, tag="rec_b")
                    with nc.allow_low_precision(reason="approx reciprocal"):
                        # denom row lives at psum partition 64; custom-DVE ops
                        # misread non-zero base partitions, so stage via sbuf
                        nc.vector.tensor_copy(
                            d2[:].rearrange("p (a q) -> p a q", q=256),
                            ao[64:65, :].rearrange("p (a q) -> p a q", q=256)[:, 0::2, :])
                        nc.vector.reciprocal_approx_fast(out=rec[:], in_=d2[:])
                        nc.scalar.copy(rec_b[:], rec[:])
                    rep = trp.tile([128, 512], F32, tag="tr", name="rep")
                    nc.tensor.matmul(rep[:], ones_sel[:], rec_b[:],
                                     start=True, stop=True)
                    rep_sb = rcp.tile([128, 512], BF, tag="rep_sb")
                    if DEBUG and m == 0 and p == 0:
                        dbg_rep_sb = stg.tile([128, 1024], F32, tag="dbgrep", name="dbgrep")
                        nc.vector.tensor_copy(dbg_rep_sb[:, 0:512], rep[:])
                        nc.vector.tensor_copy(dbg_rep_sb[0:1, 512:1024][:, 0:512], rec[:])
                        nc.sync.dma_start(dbg_rep[:], dbg_rep_sb[:])
                    with nc.allow_low_precision(reason="bf16 attention out"):
                        nc.scalar.copy(rep_sb[:], rep[:])
                        nc.vector.tensor_tensor(
                            out=AOT[p][0:64, m * 256:(m + 1) * 256],
                            in0=ao[0:64, 0:256], in1=rep_sb[0:64, 0:256],
                            op=ALU.mult)
                        nc.vector.tensor_tensor(
                            out=AOT[p][64:128, m * 256:(m + 1) * 256],
                            in0=ao[0:64, 512:768], in1=rep_sb[0:64, 256:512],
                            op=ALU.mult)
                # O-projection for the two s-chunks of this q-tile
                for i, scl in enumerate((2 * m, 2 * m + 1)):
                    op = bigp.tile([128, 1024], F32, tag="big", name="op")
                    for nb in range(2):
                        for cc in range(2):
                            nc.tensor.matmul(
                                op[:, nb * 512:(nb + 1) * 512],
                                AOT[cc][:, scl * 128:(scl + 1) * 128],
                                wo_t[:, cc, nb * 512:(nb + 1) * 512],
                                start=(cc == 0), stop=(cc == 1))
                    outs = stg.tile([128, 1024], BF, tag="stage")
                    with nc.allow_low_precision(reason="bf16 output"):
                        if i == 0:
                            nc.scalar.copy(outs[:], op[:])
                        else:
                            nc.vector.tensor_copy(outs[:], op[:])
                    nc.sync.dma_start(cc_in[scl * 128:(scl + 1) * 128, :], outs[:])
                # per-qtile reduce-scatter (256 rows -> 64 rows)
                nc.gpsimd.collective_compute(
                    "ReduceScatter", ALU.add, replica_groups=groups,
                    ins=[cc_in[256 * m:256 * (m + 1), :]],
                    outs=[cc_out[64 * m:64 * (m + 1), :]])
                nc.sync.dma_start(y_d[64 * m:64 * (m + 1), :],
                                  cc_out[64 * m:64 * (m + 1), :])

            DBG = {}
            for sc in range(NSC):
                phase_a(sc)
                if sc % 2 == 1:
                    phase_b(sc // 2)
            if DEBUG:
                nc.sync.dma_start(dbg_qtkt[:], QTKT[:].rearrange("p a s -> p (a s)"))
                nc.sync.dma_start(dbg_vo[:], VO[:].rearrange("p a b c -> p (a b c)"))
                for i in range(2):
                    nc.sync.dma_start(dbg_aot[:, i * S:(i + 1) * S], AOT[i][:])
                if "pt" in DBG:
                    nc.sync.dma_start(dbg_pt[:], DBG["pt"][:].rearrange("p a b c -> p (a b c)"))
                if "st" in DBG:
                    nc.sync.dma_start(dbg_st[:], DBG["st"][:])

    nc.compile()
    return nc


def _get_compiled():
    global _compiled
    if _compiled is None:
        _compiled = _build()
    return _compiled


def kernel(x, Wq, Wk, Wv, Wo, token_positions):
    from concourse.bass_utils import run_bass_kernel_spmd
    import ml_dtypes

    nc = _get_compiled()
    BF = ml_dtypes.bfloat16

    x = np.asarray(x, np.float32)
    Wq = np.asarray(Wq, np.float32)
    Wk = np.asarray(Wk, np.float32)
    Wv = np.asarray(Wv, np.float32)
    Wo = np.asarray(Wo, np.float32)
    pos = np.asarray(token_positions).astype(np.float32)  # [S]

    # rotate-half permutation within each head: [evens, odds]
    perm = np.concatenate([np.arange(0, DK, 2), np.arange(1, DK, 2)])
    inv_freq = (ROPE_THETA ** (-np.arange(0, DK, 2, dtype=np.float64) / DK))
    ang = pos[:, None].astype(np.float64) * inv_freq[None, :]  # [S, 32]
    cosT = np.cos(ang).astype(np.float32)  # [S, 32]
    sinT = np.sin(ang).astype(np.float32)
    # [S, 32] -> [128, NSC*32]: partition = s within chunk, chunk-major free
    def tab(t):
        return np.ascontiguousarray(
            t.reshape(NSC, 128, 32).transpose(1, 0, 2).reshape(128, NSC * 32)
        ).astype(BF)
    cosb, sinb, nsinb = tab(cosT), tab(sinT), tab(-sinT)

    kl = np.arange(128)[:, None]
    ql = np.arange(256)[None, :]
    m0 = (kl <= ql).astype(np.float32)
    m1 = (kl + 128 <= ql).astype(np.float32)
    msk = np.ascontiguousarray(np.concatenate([m0, m1], axis=1)).astype(BF)

    in_maps = []
    for c in range(8):
        b, g = c // 4, c % 4
        heads = range(HC * g, HC * (g + 1))
        rowsel = np.concatenate([h * DK + perm for h in heads])
        block = slice(E * g, E * (g + 1))
        wq = Wq[rowsel, :].T  # [1024, 256] perm'd output features
        wk = Wk[rowsel, :].T
        wqk = np.ascontiguousarray(np.concatenate([wq, wk], axis=1)).astype(BF)
        in_maps.append({
            "xt": np.ascontiguousarray(x[b].T).astype(BF),
            "wqk": wqk,
            "wv": np.ascontiguousarray(Wv[block, :].T).astype(BF),
            "wo": np.ascontiguousarray(Wo[:, block].T).astype(BF),
            "cost": cosb, "sint": sinb, "nsint": nsinb, "msk": msk,
        })

    res = run_bass_kernel_spmd(nc, in_maps, core_ids=list(range(8)))

    out = np.empty((B, S, D), np.float32)
    for b in range(B):
        for r in range(4):
            shard = np.asarray(res.results[4 * b + r]["y"]).astype(np.float32)
            for m in range(8):
                out[b, 256 * m + 64 * r: 256 * m + 64 * (r + 1), :] = \
                    shard[64 * m:64 * (m + 1), :]
    return out


# revision 17
# speedup vs baseline: 2.2175x; 1.3858x over previous
"""Causal multi-head self-attention (RoPE) Trainium2 Bass kernel.

Sharding: 8 cores = 2 batches x 4 head-groups (4 heads each).
Per core: QKV projections for its head slice, RoPE, causal flash-style
attention (S^T orientation, ones-row softmax denominator), O-projection
partial, then a per-qtile ReduceScatter (bf16) over the 4 cores of each
batch.

Fast path vs v1: bf16 operands everywhere (fp32 PSUM accumulation),
host-side x transpose + cos/sin tables, phase A/B interleaving to keep
the PE warm (HAM), row-tiled K=64 S^T matmuls (two heads concurrent in
PE quadrants), exp on [128,1024] psum tiles, multiplicative causal mask
on GPSIMD, reciprocal_approx_fast + K=1 broadcast matmul for softmax
denominators.
"""

import sys, math

sys.path.insert(0, '/opt/trn_rl_repo')
import numpy as np

B, S, D, H, DK = 2, 2048, 1024, 16, 64
HC = 4            # heads per core
E = HC * DK       # 256 local projection width
NSC = S // 128    # 16 s-chunks
NQT = S // 256    # 8 q-tiles of 256
ROPE_THETA = 10000.0

_compiled = None
import os
DEBUG = bool(int(os.environ.get("KDEBUG", "0")))


def _build():
    import concourse.bass as bass
    import concourse.tile as tile
    from concourse import bacc, mybir
    from concourse.masks import make_identity

    F32 = mybir.dt.float32
    F32R = mybir.dt.float32r
    BF = mybir.dt.bfloat16
    AF = mybir.ActivationFunctionType
    ALU = mybir.AluOpType

    nc = bacc.Bacc()

    xt_d = nc.dram_tensor("xt", [D, S], BF, kind="ExternalInput")
    wqk_d = nc.dram_tensor("wqk", [D, 2 * E], BF, kind="ExternalInput")
    wv_d = nc.dram_tensor("wv", [D, E], BF, kind="ExternalInput")
    wo_d = nc.dram_tensor("wo", [E, D], BF, kind="ExternalInput")
    cos_d = nc.dram_tensor("cost", [128, NSC * 32], BF, kind="ExternalInput")
    sin_d = nc.dram_tensor("sint", [128, NSC * 32], BF, kind="ExternalInput")
    nsin_d = nc.dram_tensor("nsint", [128, NSC * 32], BF, kind="ExternalInput")
    msk_d = nc.dram_tensor("msk", [128, 2 * 256], BF, kind="ExternalInput")
    y_d = nc.dram_tensor("y", [512, D], BF, kind="ExternalOutput")
    if DEBUG:
        dbg_qtkt = nc.dram_tensor("dbg_qtkt", [128, 4 * S], BF, kind="ExternalOutput")
        dbg_vo = nc.dram_tensor("dbg_vo", [128, NSC * HC * 65], BF, kind="ExternalOutput")
        dbg_aot = nc.dram_tensor("dbg_aot", [128, 2 * S], BF, kind="ExternalOutput")
        dbg_pt = nc.dram_tensor("dbg_pt", [128, 2 * NSC * 256], BF, kind="ExternalOutput")
        dbg_st = nc.dram_tensor("dbg_st", [128, 1024], mybir.dt.float32, kind="ExternalOutput")
        dbg_ao = nc.dram_tensor("dbg_ao", [65, 1024], mybir.dt.float32, kind="ExternalOutput")
        dbg_rep = nc.dram_tensor("dbg_rep", [128, 1024], mybir.dt.float32, kind="ExternalOutput")
    cc_in = nc.dram_tensor("cc_in", [S, D], BF)
    cc_out = nc.dram_tensor("cc_out", [512, D], BF)
    groups = [[0, 1, 2, 3], [4, 5, 6, 7]]

    with tile.TileContext(nc) as tc:
        with (
            tc.tile_pool(name="const", bufs=1) as cp,
            tc.tile_pool(name="persist", bufs=1) as bp,
            tc.tile_pool(name="rope", bufs=2) as rp,
            tc.tile_pool(name="pt", bufs=2) as ptp,
            tc.tile_pool(name="rec", bufs=2) as rcp,
            tc.tile_pool(name="stage", bufs=3) as stg,
            tc.tile_pool(name="big", bufs=2, space="PSUM") as bigp,
            tc.tile_pool(name="trp", bufs=2, space="PSUM") as trp,
            tc.tile_pool(name="aop", bufs=2, space="PSUM") as aop,
        ):
            # ---- constants / weights (bf16)
            wqk_t = cp.tile([128, 8, 2 * E], BF, tag="wqk")
            wv_t = cp.tile([128, 8, E], BF, tag="wv")
            wo_t = cp.tile([128, 2, D], BF, tag="wo")
            for c2 in range(4):
                nc.gpsimd.dma_start(
                    wqk_t[:, 2 * c2:2 * c2 + 2, :],
                    wqk_d.rearrange("(c p) e -> p c e", p=128)[:, 2 * c2:2 * c2 + 2, :])
            nc.gpsimd.dma_start(wv_t[:], wv_d.rearrange("(c p) e -> p c e", p=128))
            nc.gpsimd.dma_start(wo_t[:], wo_d.rearrange("(c p) e -> p c e", p=128))
            cost = cp.tile([128, NSC, 32], BF, tag="cost")
            sint = cp.tile([128, NSC, 32], BF, tag="sint")
            nsint = cp.tile([128, NSC, 32], BF, tag="nsint")
            nc.sync.dma_start(cost[:].rearrange("p c f -> p (c f)"), cos_d[:])
            nc.sync.dma_start(sint[:].rearrange("p c f -> p (c f)"), sin_d[:])
            nc.sync.dma_start(nsint[:].rearrange("p c f -> p (c f)"), nsin_d[:])
            masks = cp.tile([128, 2, 256], BF, tag="masks")
            nc.sync.dma_start(masks[:].rearrange("p a b -> p (a b)"), msk_d[:])
            ident = cp.tile([128, 128], BF, tag="ident")
            make_identity(nc, ident[:])
            ones_sel = cp.tile([1, 128], BF, tag="ones_sel")
            nc.vector.memset(ones_sel[:], 1.0)

            # x^T resident in SBUF: [d-part, d-chunk, s]
            xt = bp.tile([128, 8, S], BF, tag="xt")
            for sc2 in range(8):
                nc.sync.dma_start(
                    xt[:, :, sc2 * 256:(sc2 + 1) * 256],
                    xt_d.rearrange("(c p) s -> p c s",
                                   p=128)[:, :, sc2 * 256:(sc2 + 1) * 256])

            # persistent activations
            # QTKT: [p, 4, S]: 0=Q feats 0:128 (heads 0,1), 1=Q feats 128:256,
            #                  2=K feats 0:128, 3=K feats 128:256
            QTKT = bp.tile([128, 4, S], BF, tag="qtkt")
            VO = bp.tile([128, NSC, HC, 65], BF, tag="vo")
            AOT = [bp.tile([128, S], BF, tag=f"aot{i}", name=f"aot{i}")
                   for i in range(2)]
            nc.vector.memset(VO[:, :, :, 64:65], 1.0)

            def phase_a(sc):
                qkp = bigp.tile([128, 1024], F32, tag="big", name="qkp")
                for dc in range(8):
                    nc.tensor.matmul(qkp[:, 0:512], xt[:, dc, sc * 128:(sc + 1) * 128],
                                     wqk_t[:, dc, :], start=(dc == 0), stop=(dc == 7))
                    nc.tensor.matmul(qkp[:, 512:768], xt[:, dc, sc * 128:(sc + 1) * 128],
                                     wv_t[:, dc, :], start=(dc == 0), stop=(dc == 7))
                # RoPE on q|k together: [128, 512]
                cosb = cost[:, sc, :].rearrange("p (a f) -> p a f", a=1) \
                    .to_broadcast([128, 16, 32])
                sinb = sint[:, sc, :].rearrange("p (a f) -> p a f", a=1) \
                    .to_broadcast([128, 8, 32])
                nsinb = nsint[:, sc, :].rearrange("p (a f) -> p a f", a=1) \
                    .to_broadcast([128, 8, 32])
                t2 = rp.tile([128, 512], BF, tag="t2")
                u2 = rp.tile([128, 512], BF, tag="u2")
                src = qkp[:, 0:512]
                with nc.allow_low_precision(reason="bf16 rope"):
                    nc.vector.tensor_tensor(
                        out=t2[:].rearrange("p (a f) -> p a f", f=32),
                        in0=src.rearrange("p (a f) -> p a f", f=32),
                        in1=cosb, op=ALU.mult)
                    s4 = src.rearrange("p (h two f) -> p h two f", two=2, f=32)
                    u4 = u2[:].rearrange("p (h two f) -> p h two f", two=2, f=32)
                    nc.vector.tensor_tensor(out=u4[:, :, 0, :], in0=s4[:, :, 1, :],
                                            in1=nsinb, op=ALU.mult)
                    nc.vector.tensor_tensor(out=u4[:, :, 1, :], in0=s4[:, :, 0, :],
                                            in1=sinb, op=ALU.mult)
                    # t2 += u2 (bf16 psum transpose cannot accumulate)
                    nc.vector.tensor_tensor(out=t2[:], in0=t2[:], in1=u2[:],
                                            op=ALU.add)
                tr = trp.tile([128, 512], BF, tag="tr", name="tr_a")
                for q in range(4):
                    nc.tensor.transpose(tr[:, q * 128:(q + 1) * 128],
                                        t2[:, q * 128:(q + 1) * 128], ident[:])
                with nc.allow_low_precision(reason="bf16 store"):
                    nc.vector.tensor_copy(
                        QTKT[:, :, sc * 128:(sc + 1) * 128],
                        tr[:].rearrange("p (a f) -> p a f", f=128))
                    # V into 65-wide head groups
                    nc.vector.tensor_copy(
                        VO[:, sc, :, 0:64],
                        qkp[:, 512:768].rearrange("p (h f) -> p h f", f=64))

            def phase_b(m):
                nkc = 2 * m + 2
                dctx = {}
                for p in range(2):  # head pair: heads (2p, 2p+1) locally
                    pt = ptp.tile([128, 2, NSC, 256], BF, tag="pt")
                    ao = aop.tile([65, 512], F32, tag="ao")
                    if DEBUG and m == 0 and p == 0:
                        DBG["pt"] = pt
                    for kc2 in range(0, nkc, 2):
                        st = bigp.tile([128, 1024], F32, tag="big", name="st")
                        for j in range(2):
                            kc = kc2 + j
                            # row-tiled pair: head 2p on rows 0:64 -> bank0,
                            # head 2p+1 on rows 64:128 -> bank1 (concurrent)
                            nc.tensor.matmul(
                                st[:, j * 256:(j + 1) * 256],
                                QTKT[0:64, 2 + p, kc * 128:(kc + 1) * 128],
                                QTKT[0:64, p, m * 256:(m + 1) * 256],
                                start=True, stop=True)
                            nc.tensor.matmul(
                                st[:, 512 + j * 256:512 + (j + 1) * 256],
                                QTKT[64:128, 2 + p, kc * 128:(kc + 1) * 128],
                                QTKT[64:128, p, m * 256:(m + 1) * 256],
                                start=True, stop=True)
                        with nc.allow_low_precision(reason="bf16 probs"):
                            nc.scalar.activation(
                                pt[:, :, kc2:kc2 + 2, :],
                                st[:].rearrange("p (h j q) -> p h j q", h=2, q=256),
                                AF.Exp, scale=1.0 / math.sqrt(DK))
                    # causal mask on the diagonal chunk-pair (multiplicative)
                    with nc.allow_low_precision(reason="bf16 mask"):
                        nc.vector.tensor_tensor(
                            out=pt[:, :, 2 * m:2 * m + 2, :],
                            in0=pt[:, :, 2 * m:2 * m + 2, :],
                            in1=masks[:].rearrange("p a b -> p () a b")
                                .to_broadcast([128, 2, 2, 256]),
                            op=ALU.mult)
                    # one accumulation group at a time per bank (interleaved
                    # groups in a shared bank corrupt the first group)
                    for h01 in range(2):
                        for kc in range(nkc):
                            nc.tensor.matmul(ao[0:65, h01 * 256:h01 * 256 + 256],
                                             VO[:, kc, 2 * p + h01, :],
                                             pt[:, h01, kc, :],
                                             start=(kc == 0), stop=(kc == nkc - 1))
                    # denominator row -> sbuf bf16 (custom-DVE ops misread psum
                    # base partition 64, and the rep matmul needs sbuf moving)
                    d2 = rcp.tile([1, 512], BF, tag="d2", name=f"d2_{p}")
                    with nc.allow_low_precision(reason="bf16 denom"):
                        nc.vector.tensor_copy(d2[:], ao[64:65, :])
                    dctx[p] = (ao, d2)
                # normalization: broadcast denom via K=1 matmul, then one
                # reciprocal + multiply; rep MMs issued after both pairs' PV
                # so the d2 copies are long done (no PE stall)
                reps = []
                for p in range(2):
                    ao, d2 = dctx[p]
                    rep = trp.tile([128, 512], F32, tag="tr", name="rep")
                    nc.tensor.matmul(rep[:], ones_sel[:], d2[:],
                                     start=True, stop=True)
                    reps.append(rep)
                for p in range(2):
                    ao, d2 = dctx[p]
                    rep_sb = rcp.tile([128, 512], F32, tag="rep_sb")
                    with nc.allow_low_precision(reason="bf16 attention out"):
                        nc.vector.reciprocal_approx_fast(out=rep_sb[:],
                                                         in_=reps[p][:])
                        nc.vector.tensor_tensor(
                            out=AOT[p][0:64, m * 256:(m + 1) * 256],
                            in0=ao[0:64, 0:256], in1=rep_sb[0:64, 0:256],
                            op=ALU.mult)
                        nc.vector.tensor_tensor(
                            out=AOT[p][64:128, m * 256:(m + 1) * 256],
                            in0=ao[0:64, 256:512], in1=rep_sb[0:64, 256:512],
                            op=ALU.mult)

            def oproj_rs(m):
                # O-projection for the two s-chunks of this q-tile
                for i, scl in enumerate((2 * m, 2 * m + 1)):
                    op = bigp.tile([128, 1024], F32, tag="big", name="op")
                    for nb in range(2):
                        for cc in range(2):
                            nc.tensor.matmul(
                                op[:, nb * 512:(nb + 1) * 512],
                                AOT[cc][:, scl * 128:(scl + 1) * 128],
                                wo_t[:, cc, nb * 512:(nb + 1) * 512],
                                start=(cc == 0), stop=(cc == 1))
                    outs = stg.tile([128, 1024], BF, tag="stage")
                    with nc.allow_low_precision(reason="bf16 output"):
                        if i == 0:
                            nc.scalar.copy(outs[:], op[:])
                        else:
                            nc.vector.tensor_copy(outs[:], op[:])
                    nc.sync.dma_start(cc_in[scl * 128:(scl + 1) * 128, :], outs[:])
                # per-qtile reduce-scatter (256 rows -> 64 rows)
                nc.gpsimd.collective_compute(
                    "ReduceScatter", ALU.add, replica_groups=groups,
                    ins=[cc_in[256 * m:256 * (m + 1), :]],
                    outs=[cc_out[64 * m:64 * (m + 1), :]])
                nc.sync.dma_start(y_d[64 * m:64 * (m + 1), :],
                                  cc_out[64 * m:64 * (m + 1), :])

            DBG = {}
            for m in range(NQT):
                phase_a(2 * m)
                phase_a(2 * m + 1)
                if m >= 1:
                    oproj_rs(m - 1)
                phase_b(m)
            oproj_rs(NQT - 1)
            if DEBUG:
                nc.sync.dma_start(dbg_qtkt[:], QTKT[:].rearrange("p a s -> p (a s)"))
                nc.sync.dma_start(dbg_vo[:], VO[:].rearrange("p a b c -> p (a b c)"))
                for i in range(2):
                    nc.sync.dma_start(dbg_aot[:, i * S:(i + 1) * S], AOT[i][:])
                if "pt" in DBG:
                    nc.sync.dma_start(dbg_pt[:], DBG["pt"][:].rearrange("p a b c -> p (a b c)"))
                if "st" in DBG:
                    nc.sync.dma_start(dbg_st[:], DBG["st"][:])

    nc.compile()
    return nc


def _get_compiled():
    global _compiled
    if _compiled is None:
        _compiled = _build()
    return _compiled


def kernel(x, Wq, Wk, Wv, Wo, token_positions):
    from concourse.bass_utils import run_bass_kernel_spmd
    import ml_dtypes

    nc = _get_compiled()
    BF = ml_dtypes.bfloat16

    x = np.asarray(x, np.float32)
    Wq = np.asarray(Wq, np.float32)
    Wk = np.asarray(Wk, np.float32)
    Wv = np.asarray(Wv, np.float32)
    Wo = np.asarray(Wo, np.float32)
    pos = np.asarray(token_positions).astype(np.float32)  # [S]

    # rotate-half permutation within each head: [evens, odds]
    perm = np.concatenate([np.arange(0, DK, 2), np.arange(1, DK, 2)])
    inv_freq = (ROPE_THETA ** (-np.arange(0, DK, 2, dtype=np.float64) / DK))
    ang = pos[:, None].astype(np.float64) * inv_freq[None, :]  # [S, 32]
    cosT = np.cos(ang).astype(np.float32)  # [S, 32]
    sinT = np.sin(ang).astype(np.float32)
    # [S, 32] -> [128, NSC*32]: partition = s within chunk, chunk-major free
    def tab(t):
        return np.ascontiguousarray(
            t.reshape(NSC, 128, 32).transpose(1, 0, 2).reshape(128, NSC * 32)
        ).astype(BF)
    cosb, sinb, nsinb = tab(cosT), tab(sinT), tab(-sinT)

    kl = np.arange(128)[:, None]
    ql = np.arange(256)[None, :]
    m0 = (kl <= ql).astype(np.float32)
    m1 = (kl + 128 <= ql).astype(np.float32)
    msk = np.ascontiguousarray(np.concatenate([m0, m1], axis=1)).astype(BF)

    in_maps = []
    for c in range(8):
        b, g = c // 4, c % 4
        heads = range(HC * g, HC * (g + 1))
        rowsel = np.concatenate([h * DK + perm for h in heads])
        block = slice(E * g, E * (g + 1))
        wq = Wq[rowsel, :].T  # [1024, 256] perm'd output features
        wk = Wk[rowsel, :].T
        wqk = np.ascontiguousarray(np.concatenate([wq, wk], axis=1)).astype(BF)
        in_maps.append({
            "xt": np.ascontiguousarray(x[b].T).astype(BF),
            "wqk": wqk,
            "wv": np.ascontiguousarray(Wv[block, :].T).astype(BF),
            "wo": np.ascontiguousarray(Wo[:, block].T).astype(BF),
            "cost": cosb, "sint": sinb, "nsint": nsinb, "msk": msk,
        })

    res = run_bass_kernel_spmd(nc, in_maps, core_ids=list(range(8)))

    out = np.empty((B, S, D), np.float32)
    for b in range(B):
        for r in range(4):
            shard = np.asarray(res.results[4 * b + r]["y"]).astype(np.float32)
            for m in range(8):
                out[b, 256 * m + 64 * r: 256 * m + 64 * (r + 1), :] = \
                    shard[64 * m:64 * (m + 1), :]
    return out
